# revision 21
# baseline (speedup 1.0000x reference)
"""DeepSeekV3 MLA attention on 8 Trainium2 NeuronCores.

Sharding: DP2 (batch) x TP4 (heads). Core c handles batch c//4 and heads
[4*(c%4), 4*(c%4)+4). Each core computes a partial output (its heads' slice
of the row-parallel wo matmul); the host sums the 4 partials per batch.

All on-device tensors use transposed [dim, seq] layouts so every matmul
contracts along the partition axis. The wq columns are reordered host-side so
the RoPE lo/hi halves of all 4 heads land as partition-aligned [128, s] tiles.

v2 changes vs baseline:
- AV computed in [VD, queries] layout (v as stationary, probs as moving
  operand): 160 wide matmuls instead of 544 narrow ones, and the output
  feeds the wo matmul directly with no PE transposes.
- Softmax denominators via a DVE running sum over key tiles + a ones^T
  matmul partition reduction; normalization applied once per (head, chunk)
  on the attention output.
- RoPE score matmuls packed two heads per PE pass via row tiling
  (K=64 upper/lower halves run concurrently).
- wkv_b / wo weights preload during phase 1 and the post-AllGather latent
  pulls land in persistent SBUF so the per-chunk pipeline starts without a
  phase-transition stall.
- Output stored transposed [E, S] in bf16 (host transposes + accumulates).
- Dummy 16-byte AllGather at kernel start absorbs the collective bootstrap
  barrier so the real AllGather completes sooner.
"""

import math
import numpy as np
import ml_dtypes

import concourse.bass as bass
import concourse.mybir as mybir
import concourse.tile as tile
from concourse import bacc
from concourse.bass_utils import run_bass_kernel_spmd

B, S, E, H = 2, 2048, 2048, 16
NOPE, ROPE, VD = 128, 64, 128
QH = NOPE + ROPE  # 192
LORA = 512
ROPE_THETA = 10000.0
EPS = 1e-6

N_CORES = 8
HPC = H // 4          # heads per core = 4
SM_SCALE = QH ** -0.5

F32 = mybir.dt.float32
BF16 = mybir.dt.bfloat16

DT_PROJ = BF16   # q/kva projection inputs (xT, wq, wkva)
DT_SC = BF16     # scores inputs (qT, kT)
DT_KVB = BF16    # kvb matmul inputs (ckv_nT, wkvb)
DT_V = BF16      # AV inputs (V lhsT / probs rhs)
DT_WO = BF16     # wo matmul inputs (att2, wo)

NP_OF = {BF16: ml_dtypes.bfloat16, F32: np.float32}

SCH = 512          # seq chunk
NSC = S // SCH     # 4
NSB = S // 128     # 16
NE = E // 128      # 16
NC_ = LORA // 128  # 4


def build_nc():
    nc = bacc.Bacc("TRN2", target_bir_lowering=False, debug=False)

    xt = nc.dram_tensor("xt", [E, S], DT_PROJ, kind="ExternalInput")
    xq = nc.dram_tensor("xq", [E, SCH], DT_PROJ, kind="ExternalInput")
    cosq = nc.dram_tensor("cosq", [128, SCH], BF16, kind="ExternalInput")
    sinq = nc.dram_tensor("sinq", [128, SCH], BF16, kind="ExternalInput")
    ccin = nc.dram_tensor("ccin", [LORA + ROPE, SCH], BF16)
    ccgat = nc.dram_tensor("ccgat", [4, LORA + ROPE, SCH], BF16)
    agp_in = nc.dram_tensor("agp_in", [1, 16], BF16)
    agp_out = nc.dram_tensor("agp_out", [4, 16], BF16)
    wq = nc.dram_tensor("wq", [E, 6 * 128], DT_PROJ, kind="ExternalInput")
    wkva = nc.dram_tensor("wkva", [E, LORA + ROPE], DT_PROJ, kind="ExternalInput")
    wkvbk = nc.dram_tensor("wkvbk", [LORA, HPC * NOPE], DT_KVB, kind="ExternalInput")
    wkvbv = nc.dram_tensor("wkvbv", [LORA, HPC * VD], DT_KVB, kind="ExternalInput")
    wo = nc.dram_tensor("wo", [HPC * VD, E], DT_WO, kind="ExternalInput")
    kvsc = nc.dram_tensor("kvsc", [128, NC_], F32, kind="ExternalInput")
    cos4 = nc.dram_tensor("cos4", [128, S], BF16, kind="ExternalInput")
    sin4 = nc.dram_tensor("sin4", [128, S], BF16, kind="ExternalInput")
    trilm = nc.dram_tensor("trilm", [128, 128], BF16, kind="ExternalInput")
    out = nc.dram_tensor("out", [E, S], BF16, kind="ExternalOutput")

    with tile.TileContext(nc) as tc:
        with (
            tc.tile_pool(name="persist", bufs=1) as pp,
            tc.tile_pool(name="tables", bufs=1) as tbl,
        ):
            tril_t = tbl.tile([128, 128], BF16)
            kvsc_t = tbl.tile([128, NC_], F32)
            ones_t = tbl.tile([128, 1], BF16)
            onesr_t = tbl.tile([1, 128], F32)
            onesrb_t = tbl.tile([1, 128], BF16)
            eps_t = tbl.tile([1, 1], F32)
            nc.sync.dma_start(tril_t[:], trilm[:])
            nc.sync.dma_start(kvsc_t[:], kvsc[:])
            nc.vector.memset(ones_t[:], 1.0)
            nc.vector.memset(onesr_t[:], 1.0)
            nc.vector.memset(onesrb_t[:], 1.0)
            nc.vector.memset(eps_t[:], EPS)

            # persistent per-chunk activations (precise deps -> overlap)
            q_nope = [[pp.tile([128, SCH], DT_SC, name=f"q_nope{h}_{c}")
                       for c in range(NSC)] for h in range(HPC)]
            # q rope packed 2 heads per tile: rows 0:64 head 2p, 64:128
            # head 2p+1 (each head: lo 32 | hi 32)
            q_ropep = [[pp.tile([128, SCH], DT_SC, name=f"q_ropep{p}_{c}")
                        for c in range(NSC)] for p in range(2)]
            # k rope duplicated rows 0:64 == 64:128 for row-tiled rope MMs
            krd = [pp.tile([128, SCH], DT_SC, name=f"krd{c}")
                   for c in range(NSC)]
            ckv_n = [[pp.tile([128, SCH], DT_KVB, name=f"ckv_n{c}_{s}")
                      for s in range(NSC)] for c in range(NC_)]
            k_nope = [[pp.tile([128, SCH], DT_SC, name=f"k_nope{h}_{c}")
                       for c in range(NSC)] for h in range(HPC)]
            v_t = [[pp.tile([128, 4, VD], DT_V, name=f"v{h}_{c}")
                    for c in range(NSC)] for h in range(HPC)]
            wbk_t = pp.tile([128, NC_, HPC * NOPE], DT_KVB)
            wbv_t = pp.tile([128, NC_, HPC * VD], DT_KVB)

            # ---------- Phase 1: q + kva projections ----------
            with (
                tc.tile_pool(name="p1w", bufs=1) as p1w,
                tc.tile_pool(name="p1rope", bufs=2) as p1r,
                tc.tile_pool(name="p1x", bufs=26) as p1x,
                tc.tile_pool(name="p1tmp", bufs=2) as p1tmp,
                tc.tile_pool(name="ps1", bufs=8, space="PSUM") as psp,
            ):
                wq_t = p1w.tile([128, NE, 6 * 128], DT_PROJ)
                wkva_t = p1w.tile([128, NE, LORA + ROPE], DT_PROJ)
                cos_t = p1w.tile([128, S], BF16)
                sin_t = p1w.tile([128, S], BF16)
                # kva quarter inputs land first so the sharded kva pass can
                # start immediately and trigger the AllGather early
                xqs = [p1x.tile([128, SCH], DT_PROJ, name=f"xqs_{e}",
                                tag="xts") for e in range(NE)]
                for e in range(NE):
                    nc.sync.dma_start(xqs[e][:], xq[e * 128:(e + 1) * 128, :])
                    nc.sync.dma_start(wkva_t[:, e, :],
                                      wkva[e * 128:(e + 1) * 128, :])
                cosq_t = p1w.tile([128, SCH], BF16)
                sinq_t = p1w.tile([128, SCH], BF16)
                nc.sync.dma_start(cosq_t[:], cosq[:])
                nc.sync.dma_start(sinq_t[:], sinq[:])

                # q-projection inputs (first chunk + weights)
                xts0 = [p1x.tile([128, SCH], DT_PROJ, name=f"xts0_{e}",
                                 tag="xts") for e in range(NE)]
                for e in range(NE):
                    nc.sync.dma_start(xts0[e][:], xt[e * 128:(e + 1) * 128,
                                                     0:SCH])
                    nc.sync.dma_start(wq_t[:, e, :],
                                      wq[e * 128:(e + 1) * 128, :])
                nc.sync.dma_start(cos_t[:], cos4[:])
                nc.sync.dma_start(sin_t[:], sin4[:])

                # phase-2 weight preloads (bandwidth is free here)
                nc.sync.dma_start(
                    wbk_t[:], wkvbk.rearrange("(nc p) d -> p nc d", p=128))
                nc.sync.dma_start(
                    wbv_t[:], wkvbv.rearrange("(nc p) d -> p nc d", p=128))

                # ---- sharded kva: this core's s-quarter only ----
                psB = [psp.tile([128, SCH], F32, tag="ps",
                                name=f"psBq_{c}") for c in range(NC_)]
                psK = psp.tile([64, SCH], F32, tag="ps", name="psKq")
                for e in range(NE):
                    for c in range(NC_):
                        nc.tensor.matmul(
                            psB[c][:], wkva_t[:, e, c * 128:(c + 1) * 128],
                            xqs[e][:], start=(e == 0), stop=(e == NE - 1))
                    nc.tensor.matmul(psK[:], wkva_t[:, e, LORA:LORA + ROPE],
                                     xqs[e][:], start=(e == 0),
                                     stop=(e == NE - 1))
                ckv_q = [p1r.tile([128, SCH], BF16, name=f"ckv_q{c}",
                                  tag=f"ckvq{c}", bufs=1) for c in range(NC_)]
                for c in range(NC_):
                    nc.scalar.copy(ckv_q[c][:], psB[c][:])
                kpe_q = p1r.tile([64, SCH], BF16, tag="kpeq", bufs=1)
                nc.scalar.copy(kpe_q[:], psK[:])
                # RMSNorm on the quarter
                sqq = [p1tmp.tile([128, SCH], BF16, name=f"sqq{c}",
                                  tag=f"sqq{c}", bufs=1) for c in range(NC_)]
                for c in range(NC_):
                    nc.vector.tensor_tensor(sqq[c][:], ckv_q[c][:], ckv_q[c][:],
                                            mybir.AluOpType.mult)
                # first q-projection e-slice keeps the PE busy while the
                # RMSNorm square chain (ACT+DVE) produces sqq
                psA0 = [psp.tile([128, SCH], F32, tag="ps",
                                 name=f"psA0_{d}") for d in range(6)]
                for d in range(6):
                    nc.tensor.matmul(
                        psA0[d][:], wq_t[:, 0, d * 128:(d + 1) * 128],
                        xts0[0][:], start=True, stop=False)
                pss = psp.tile([1, SCH], F32, tag="ps", name="ssqq")
                for c in range(NC_):
                    nc.tensor.matmul(pss[:], ones_t[:], sqq[c][:],
                                     start=(c == 0), stop=(c == NC_ - 1))
                s_rowq = p1tmp.tile([1, SCH], F32, tag="srq", bufs=1)
                r_rowq = p1tmp.tile([1, SCH], F32, tag="rrq", bufs=1)
                nc.scalar.activation(s_rowq[:], pss[:],
                                     mybir.ActivationFunctionType.Ln,
                                     bias=eps_t[:], scale=1.0 / LORA)
                nc.scalar.activation(r_rowq[:], s_rowq[:],
                                     mybir.ActivationFunctionType.Exp,
                                     scale=-0.5)
                r_bcq = psp.tile([128, SCH], F32, tag="ps", name="rbcq")
                nc.tensor.matmul(r_bcq[:], onesr_t[0:1, :], r_rowq[:],
                                 start=True, stop=True)
                ckv_nq = [p1r.tile([128, SCH], BF16, name=f"ckv_nq{c}",
                                   tag=f"ckvnq{c}", bufs=1)
                          for c in range(NC_)]
                for c in range(NC_):
                    nc.vector.scalar_tensor_tensor(
                        ckv_nq[c][:], ckv_q[c][:], kvsc_t[:, c:c + 1],
                        r_bcq[:], op0=mybir.AluOpType.mult,
                        op1=mybir.AluOpType.mult)
                    nc.sync.dma_start(ccin[c * 128:(c + 1) * 128, :],
                                      ckv_nq[c][:])
                # k_pe rope on the quarter
                kloq = p1r.tile([32, SCH], BF16, tag="kloq", bufs=1)
                khiq = p1r.tile([32, SCH], BF16, tag="khiq", bufs=1)
                nc.sync.dma_start(kloq[:], kpe_q[0:32, :])
                nc.sync.dma_start(khiq[:], kpe_q[32:64, :])
                kt1q = p1tmp.tile([32, SCH], BF16, tag="kt1", bufs=1)
                kt2q = p1tmp.tile([32, SCH], BF16, tag="kt2", bufs=1)
                klorq = p1r.tile([32, SCH], BF16, tag="klorq", bufs=1)
                khirq = p1r.tile([32, SCH], BF16, tag="khirq", bufs=1)
                nc.vector.tensor_tensor(kt1q[:], kloq[:], cosq_t[0:32, :],
                                        mybir.AluOpType.mult)
                nc.vector.tensor_tensor(kt2q[:], khiq[:], sinq_t[0:32, :],
                                        mybir.AluOpType.mult)
                nc.vector.tensor_tensor(klorq[:], kt1q[:], kt2q[:],
                                        mybir.AluOpType.subtract)
                nc.vector.tensor_tensor(kt1q[:], khiq[:], cosq_t[0:32, :],
                                        mybir.AluOpType.mult)
                nc.vector.tensor_tensor(kt2q[:], kloq[:], sinq_t[0:32, :],
                                        mybir.AluOpType.mult)
                nc.vector.tensor_tensor(khirq[:], kt1q[:], kt2q[:],
                                        mybir.AluOpType.add)
                nc.sync.dma_start(ccin[LORA:LORA + 32, :], klorq[:])
                nc.sync.dma_start(ccin[LORA + 32:LORA + ROPE, :], khirq[:])
                nc.gpsimd.collective_compute(
                    "AllGather", mybir.AluOpType.bypass,
                    replica_groups=[[0, 1, 2, 3], [4, 5, 6, 7]],
                    ins=[ccin[:]], outs=[ccgat[:]])

                for sc in range(NSC):
                    ssl = bass.ts(sc, SCH)
                    if sc == 0:
                        xts = xts0
                        psA = psA0
                        e0 = 1  # e == 0 already issued above
                    else:
                        xts = [p1x.tile([128, SCH], DT_PROJ,
                                        name=f"xts{sc}_{e}", tag="xts")
                               for e in range(NE)]
                        for e in range(NE):
                            nc.sync.dma_start(
                                xts[e][:], xt[e * 128:(e + 1) * 128, ssl])
                        psA = [psp.tile([128, SCH], F32, tag="ps",
                                        name=f"psA{sc}_{d}") for d in range(6)]
                        e0 = 0
                    # pass A: 6 q-projection groups, e-outer
                    for e in range(e0, NE):
                        for d in range(6):
                            nc.tensor.matmul(
                                psA[d][:], wq_t[:, e, d * 128:(d + 1) * 128],
                                xts[e][:], start=(e == 0), stop=(e == NE - 1))
                    for h in range(HPC):
                        nc.scalar.copy(q_nope[h][sc][:], psA[h][:])
                    # drain rope psum banks to SBUF first (ACT) so the next
                    # chunk's psA accumulation can reuse them promptly
                    lo_sb = p1tmp.tile([128, SCH], BF16, tag="losb",
                                       name=f"losb_{sc}")
                    hi_sb = p1tmp.tile([128, SCH], BF16, tag="hisb",
                                       name=f"hisb_{sc}")
                    nc.scalar.copy(lo_sb[:], psA[4][:])
                    nc.scalar.copy(hi_sb[:], psA[5][:])
                    # q rope: lo' = lo*cos - hi*sin ; hi' = hi*cos + lo*sin
                    t1 = p1tmp.tile([128, SCH], BF16, tag="t1")
                    t2 = p1tmp.tile([128, SCH], BF16, tag="t2")
                    t3 = p1tmp.tile([128, SCH], BF16, tag="t1",
                                    name=f"t3_{sc}")
                    t4 = p1tmp.tile([128, SCH], BF16, tag="t2",
                                    name=f"t4_{sc}")
                    qlo4 = p1r.tile([128, SCH], BF16, tag="qlo4",
                                    name=f"qlo4_{sc}")
                    qhi4 = p1r.tile([128, SCH], BF16, tag="qhi4",
                                    name=f"qhi4_{sc}")
                    nc.vector.tensor_tensor(t1[:], lo_sb[:], cos_t[:, ssl],
                                            mybir.AluOpType.mult)
                    nc.vector.tensor_tensor(t2[:], hi_sb[:], sin_t[:, ssl],
                                            mybir.AluOpType.mult)
                    nc.vector.tensor_tensor(qlo4[:], t1[:], t2[:],
                                            mybir.AluOpType.subtract)
                    nc.vector.tensor_tensor(t3[:], hi_sb[:], cos_t[:, ssl],
                                            mybir.AluOpType.mult)
                    nc.vector.tensor_tensor(t4[:], lo_sb[:], sin_t[:, ssl],
                                            mybir.AluOpType.mult)
                    nc.vector.tensor_tensor(qhi4[:], t3[:], t4[:],
                                            mybir.AluOpType.add)
                    for p in range(2):
                        for j in range(2):
                            h = 2 * p + j
                            hs = bass.ts(h, 32)
                            nc.sync.dma_start(
                                q_ropep[p][sc][64 * j:64 * j + 32, :],
                                qlo4[hs, :])
                            nc.sync.dma_start(
                                q_ropep[p][sc][64 * j + 32:64 * j + 64, :],
                                qhi4[hs, :])

                # pull the gathered, normalized ckv + roped k_pe into SBUF
                # (persistent tiles: transfers overlap the q projection)
                for sc in range(NSC):
                    for c in range(NC_):
                        nc.sync.dma_start(ckv_n[c][sc][:],
                                          ccgat[sc, c * 128:(c + 1) * 128, :])
                    nc.sync.dma_start(krd[sc][0:64, :],
                                      ccgat[sc, LORA:LORA + ROPE, :])
                    nc.sync.dma_start(krd[sc][64:128, :],
                                      ccgat[sc, LORA:LORA + ROPE, :])

            # ---------- fused per-chunk pipeline: kvb / attn / wo ----------
            with (
                tc.tile_pool(name="p2w", bufs=1) as p2w,
                tc.tile_pool(name="probs", bufs=8) as prp,
                tc.tile_pool(name="acc", bufs=4) as accp,
                tc.tile_pool(name="att2", bufs=8) as attp,
                tc.tile_pool(name="a2tmp", bufs=4) as a2t,
                tc.tile_pool(name="aout", bufs=4) as aout,
                tc.tile_pool(name="psO", bufs=4, space="PSUM") as psO,
                tc.tile_pool(name="psS", bufs=4, space="PSUM") as psS,
            ):
                wo_t = [p2w.tile([128, E], DT_WO, name=f"wo_t{h}")
                        for h in range(HPC)]
                for h in range(HPC):
                    nc.sync.dma_start(wo_t[h][:], wo[h * 128:(h + 1) * 128, :])
                def emit_kvb(qc):
                    # kvb(qc): k_nope + V (drains on ACT: it has slack here)
                    for h in range(HPC):
                        ps2 = psS.tile([128, SCH], F32, tag="sc",
                                       name=f"kvbk{qc}_{h}")
                        for c in range(NC_):
                            nc.tensor.matmul(
                                ps2[:], wbk_t[:, c, h * 128:(h + 1) * 128],
                                ckv_n[c][qc][:], start=(c == 0),
                                stop=(c == NC_ - 1))
                        nc.scalar.copy(k_nope[h][qc][:], ps2[:])
                    for sbl in range(4):
                        psv = psS.tile([128, HPC * VD], F32, tag="sc",
                                       name=f"v{qc}_{sbl}")
                        for c in range(NC_):
                            nc.tensor.matmul(
                                psv[:], ckv_n[c][qc][:, bass.ts(sbl, 128)],
                                wbv_t[:, c, :], start=(c == 0),
                                stop=(c == NC_ - 1))
                        for h in range(HPC):
                            nc.scalar.copy(v_t[h][qc][:, sbl, :],
                                           psv[:, h * VD:(h + 1) * VD])

                emit_kvb(0)
                for qc in range(NSC):
                    # ---- attention(qc), two heads per pass ----
                    # AV matmuls run one ki behind the score matmuls so the
                    # PE never waits on the exp; denominators accumulate on
                    # DVE (heads 0,2) / GpSimd (heads 1,3); all four heads'
                    # normalization chains are batched after both passes.
                    nki = 4 * qc + 4
                    att2 = [None] * HPC
                    out2h = [None] * HPC
                    acch = [None] * HPC
                    for p in range(2):
                        hA, hB = 2 * p, 2 * p + 1
                        out2 = [psO.tile([128, SCH], F32, tag="o2",
                                         name=f"o2_{qc}_{p}_{j}")
                                for j in range(2)]
                        acc = [accp.tile([128, SCH], F32, tag="acc",
                                         name=f"acc{qc}_{p}_{j}")
                               for j in range(2)]
                        out2h[hA], out2h[hB] = out2[0], out2[1]
                        acch[hA], acch[hB] = acc[0], acc[1]

                        def emit_av(ent):
                            pki, pvl, pprs = ent
                            pkc, pko = pki // 4, pki % 4
                            for j, hh in enumerate((hA, hB)):
                                nc.tensor.matmul(
                                    out2[j][:, pvl], v_t[hh][pkc][:, pko, :],
                                    pprs[j][:, pvl], start=(pki == 0),
                                    stop=(pki == nki - 1))

                        # AV runs two ki behind scores so the PE never waits
                        # on the exp (ACT runs ~1 ki behind with slack)
                        pend = []
                        for ki in range(nki):
                            kc, ko = ki // 4, ki % 4
                            js = max(0, ki - 4 * qc)
                            w = SCH - js * 128
                            vl = bass.ds(js * 128, w)
                            kb = bass.ts(ko, 128)
                            ps2h = [psS.tile([128, SCH], F32, tag="sc",
                                             name=f"sc{qc}_{p}_{ki}_{j}")
                                    for j in range(2)]
                            for j, hh in enumerate((hA, hB)):
                                nc.tensor.matmul(
                                    ps2h[j][:, vl], k_nope[hh][kc][:, kb],
                                    q_nope[hh][qc][:, vl],
                                    start=True, stop=False)
                            # rope: both heads concurrently via row tiling
                            nc.tensor.matmul(
                                ps2h[0][:, vl], krd[kc][0:64, kb],
                                q_ropep[p][qc][0:64, vl],
                                start=False, stop=True)
                            nc.tensor.matmul(
                                ps2h[1][:, vl], krd[kc][64:128, kb],
                                q_ropep[p][qc][64:128, vl],
                                start=False, stop=True)
                            if len(pend) == 2:
                                emit_av(pend.pop(0))
                            prs = []
                            for j, hh in enumerate((hA, hB)):
                                pr = prp.tile([128, SCH], DT_V, tag="pr",
                                              name=f"pr{qc}_{p}_{ki}_{j}")
                                nc.scalar.activation(
                                    pr[:, vl], ps2h[j][:, vl],
                                    mybir.ActivationFunctionType.Exp,
                                    scale=SM_SCALE)
                                if ki >= 4 * qc:
                                    jb = bass.ts(ki - 4 * qc, 128)
                                    nc.vector.tensor_tensor(
                                        pr[:, jb], pr[:, jb], tril_t[:],
                                        mybir.AluOpType.mult)
                                # denominator running sum (f32, DVE)
                                if ki == 0:
                                    nc.vector.tensor_copy(acc[j][:], pr[:])
                                else:
                                    nc.vector.tensor_tensor(
                                        acc[j][:, vl], acc[j][:, vl],
                                        pr[:, vl], mybir.AluOpType.add)
                                prs.append(pr)
                            pend.append((ki, vl, prs))
                        for ent in pend:
                            emit_av(ent)
                        # bf16 denominator rows for this pair (DVE-only, no
                        # PE involvement, lands early in the DVE queue)
                        for j, hh in enumerate((hA, hB)):
                            accb = a2t.tile([128, SCH], BF16, tag="accb",
                                            name=f"accb{qc}_{hh}")
                            nc.vector.tensor_copy(accb[:], acc[j][:])
                            acch[hh] = accb
                    accbs = acch
                    # next chunk's kvb: PE filler while the denominator
                    # chains drain on DVE
                    if qc + 1 < NSC:
                        emit_kvb(qc + 1)
                    # normalize: att2 = out2 / colsum (all heads batched)
                    recs, psds, psbcs, bcss = [], [], [], []
                    for hh in range(HPC):
                        psd = psS.tile([1, SCH], F32, tag="sc",
                                       padded_shape=[128, SCH],
                                       name=f"psd{qc}_{hh}")
                        nc.tensor.matmul(psd[:], ones_t[:], accbs[hh][:],
                                         start=True, stop=True)
                        psds.append(psd)
                    lns = []
                    for hh in range(HPC):
                        ln = a2t.tile([1, SCH], F32, tag="ln",
                                      name=f"ln{qc}_{hh}")
                        nc.scalar.activation(ln[:], psds[hh][:],
                                             mybir.ActivationFunctionType.Ln)
                        lns.append(ln)
                    for hh in range(HPC):
                        # 1/x = exp(-ln(x)) on ACT: a [1,512] DVE reciprocal
                        # is single-lane and costs ~3.3us; this is ~0.4us
                        rec = a2t.tile([1, SCH], BF16, tag="rec",
                                       name=f"rec{qc}_{hh}")
                        nc.scalar.activation(rec[:], lns[hh][:],
                                             mybir.ActivationFunctionType.Exp,
                                             scale=-1.0)
                        recs.append(rec)
                    for hh in range(HPC):
                        psbc = psS.tile([128, SCH], F32, tag="sc",
                                        name=f"psbc{qc}_{hh}")
                        nc.tensor.matmul(psbc[:], onesrb_t[0:1, :],
                                         recs[hh][:], start=True, stop=True)
                        psbcs.append(psbc)
                    for hh in range(HPC):
                        bcs = a2t.tile([128, SCH], BF16, tag="bcs",
                                       name=f"bcs{qc}_{hh}")
                        nc.scalar.copy(bcs[:], psbcs[hh][:])
                        bcss.append(bcs)
                    for hh in range(HPC):
                        at2 = attp.tile([128, SCH], DT_WO, tag="att2",
                                        name=f"att2_{qc}_{hh}")
                        nc.vector.tensor_tensor(at2[:], out2h[hh][:],
                                                bcss[hh][:],
                                                mybir.AluOpType.mult)
                        att2[hh] = at2

                    # ---- wo(qc): outT[e, q] += wo^T @ att2 ----
                    for ecg in range(4):
                        psos = [psS.tile([128, SCH], F32, tag="sc",
                                         name=f"po{qc}_{ecg}_{j}")
                                for j in range(4)]
                        for h in range(HPC):
                            for j in range(4):
                                ec = 4 * ecg + j
                                nc.tensor.matmul(
                                    psos[j][:], wo_t[h][:, bass.ts(ec, 128)],
                                    att2[h][:], start=(h == 0),
                                    stop=(h == HPC - 1))
                        for j in range(4):
                            ec = 4 * ecg + j
                            ot = aout.tile([128, SCH], BF16, tag="ot",
                                           name=f"ot{qc}_{ec}")
                            if j % 2 == 0:
                                nc.scalar.copy(ot[:], psos[j][:])
                            else:
                                nc.vector.tensor_copy(ot[:], psos[j][:])
                            nc.sync.dma_start(
                                out[bass.ts(ec, 128), bass.ts(qc, SCH)],
                                ot[:])

    nc.finalize()
    return nc


def _prep_inputs(x, wq, wkv_a, wkv_b, wo, kv_norm_scale):
    """Build the 8 per-core input dicts (numpy, host-side sharding)."""
    x = np.asarray(x, np.float32)
    wq = np.asarray(wq, np.float32)
    wkv_a = np.asarray(wkv_a, np.float32)
    wkv_b = np.asarray(wkv_b, np.float32)
    wo = np.asarray(wo, np.float32)
    kv_norm_scale = np.asarray(kv_norm_scale, np.float32)

    bf = ml_dtypes.bfloat16
    pos = np.arange(S, dtype=np.float32)
    inv = 1.0 / (ROPE_THETA ** (np.arange(0, ROPE, 2, dtype=np.float32) / ROPE))
    ang = pos[:, None] * inv  # [S, 32]
    cosT = np.cos(ang).T  # [32, S]
    sinT = np.sin(ang).T
    cos4 = np.tile(cosT, (4, 1)).astype(bf)
    sin4 = np.tile(sinT, (4, 1)).astype(bf)
    tril = (np.arange(128)[None, :] >= np.arange(128)[:, None]).astype(bf)
    kvsc = kv_norm_scale.reshape(NC_, 128).T.copy()  # [128, NC_]

    wq_r = wq.reshape(E, H, QH)
    wkv_b_r = wkv_b.reshape(LORA, H, NOPE + VD)
    wo_r = wo.reshape(H, VD, E)

    in_maps = []
    for c in range(N_CORES):
        b, hg = c // 4, c % 4
        hs = [4 * hg + j for j in range(HPC)]
        xt = np.ascontiguousarray(x[b].T).astype(NP_OF[DT_PROJ])
        r = hg  # this core's s-quarter for the sharded kva projection
        xq_loc = np.ascontiguousarray(xt[:, r * SCH:(r + 1) * SCH])
        cosq_loc = np.ascontiguousarray(cos4[:, r * SCH:(r + 1) * SCH])
        sinq_loc = np.ascontiguousarray(sin4[:, r * SCH:(r + 1) * SCH])
        # wq cols: nope h0..h3 | lo4 | hi4
        wq_loc = np.concatenate(
            [wq_r[:, h, 0:NOPE] for h in hs]
            + [np.concatenate([wq_r[:, h, NOPE:NOPE + 32] for h in hs], axis=1)]
            + [np.concatenate([wq_r[:, h, NOPE + 32:QH] for h in hs], axis=1)],
            axis=1).astype(NP_OF[DT_PROJ])
        wkvbk = np.concatenate([wkv_b_r[:, h, 0:NOPE] for h in hs],
                               axis=1).astype(NP_OF[DT_KVB])
        wkvbv = np.concatenate([wkv_b_r[:, h, NOPE:] for h in hs],
                               axis=1).astype(NP_OF[DT_KVB])
        wo_loc = np.concatenate([wo_r[h] for h in hs],
                                axis=0).astype(NP_OF[DT_WO])
        in_maps.append({
            "xt": xt,
            "xq": xq_loc,
            "cosq": cosq_loc,
            "sinq": sinq_loc,
            "wq": wq_loc,
            "wkva": wkv_a.astype(NP_OF[DT_PROJ]),
            "wkvbk": wkvbk,
            "wkvbv": wkvbv,
            "wo": wo_loc,
            "kvsc": kvsc,
            "cos4": cos4,
            "sin4": sin4,
            "trilm": tril,
        })
    return in_maps


_LAST_EXEC_NS = None


def kernel(x, wq, wkv_a, wkv_b, wo, kv_norm_scale, _trace=False):
    global _LAST_EXEC_NS
    nc = build_nc()
    in_maps = _prep_inputs(x, wq, wkv_a, wkv_b, wo, kv_norm_scale)
    res = run_bass_kernel_spmd(nc, in_maps, list(range(N_CORES)), trace=_trace)
    _LAST_EXEC_NS = res.exec_time_ns
    out = np.zeros((B, S, E), np.float32)
    for c in range(N_CORES):
        out[c // 4] += np.asarray(res.results[c]["out"],
                                  dtype=np.float32).T
    return out


# revision 23
# speedup vs baseline: 1.0507x; 1.0507x over previous
"""DeepSeekV3 MLA attention on 8 Trainium2 NeuronCores.

Sharding: DP2 (batch) x TP4 (heads). Core c handles batch c//4 and heads
[4*(c%4), 4*(c%4)+4). Each core computes a partial output (its heads' slice
of the row-parallel wo matmul); the host sums the 4 partials per batch.

All on-device tensors use transposed [dim, seq] layouts so every matmul
contracts along the partition axis. The wq columns are reordered host-side so
the RoPE lo/hi halves of all 4 heads land as partition-aligned [128, s] tiles.

v2 changes vs baseline:
- AV computed in [VD, queries] layout (v as stationary, probs as moving
  operand): 160 wide matmuls instead of 544 narrow ones, and the output
  feeds the wo matmul directly with no PE transposes.
- Softmax denominators via a DVE running sum over key tiles + a ones^T
  matmul partition reduction; normalization applied once per (head, chunk)
  on the attention output.
- RoPE score matmuls packed two heads per PE pass via row tiling
  (K=64 upper/lower halves run concurrently).
- wkv_b / wo weights preload during phase 1 and the post-AllGather latent
  pulls land in persistent SBUF so the per-chunk pipeline starts without a
  phase-transition stall.
- Output stored transposed [E, S] in bf16 (host transposes + accumulates).
- Dummy 16-byte AllGather at kernel start absorbs the collective bootstrap
  barrier so the real AllGather completes sooner.
"""

import math
import numpy as np
import ml_dtypes

import concourse.bass as bass
import concourse.mybir as mybir
import concourse.tile as tile
from concourse import bacc
from concourse.bass_utils import run_bass_kernel_spmd

B, S, E, H = 2, 2048, 2048, 16
NOPE, ROPE, VD = 128, 64, 128
QH = NOPE + ROPE  # 192
LORA = 512
ROPE_THETA = 10000.0
EPS = 1e-6

N_CORES = 8
HPC = H // 4          # heads per core = 4
SM_SCALE = QH ** -0.5

F32 = mybir.dt.float32
BF16 = mybir.dt.bfloat16

DT_PROJ = BF16   # q/kva projection inputs (xT, wq, wkva)
DT_SC = BF16     # scores inputs (qT, kT)
DT_KVB = BF16    # kvb matmul inputs (ckv_nT, wkvb)
DT_V = BF16      # AV inputs (V lhsT / probs rhs)
DT_WO = BF16     # wo matmul inputs (att2, wo)

NP_OF = {BF16: ml_dtypes.bfloat16, F32: np.float32}

SCH = 512          # seq chunk
NSC = S // SCH     # 4
NSB = S // 128     # 16
NE = E // 128      # 16
NC_ = LORA // 128  # 4


def build_nc():
    nc = bacc.Bacc("TRN2", target_bir_lowering=False, debug=False)

    xt = nc.dram_tensor("xt", [E, S], DT_PROJ, kind="ExternalInput")
    xq = nc.dram_tensor("xq", [E, SCH], DT_PROJ, kind="ExternalInput")
    cosq = nc.dram_tensor("cosq", [128, SCH], BF16, kind="ExternalInput")
    sinq = nc.dram_tensor("sinq", [128, SCH], BF16, kind="ExternalInput")
    ccin = nc.dram_tensor("ccin", [LORA + ROPE, SCH], BF16)
    ccgat = nc.dram_tensor("ccgat", [4, LORA + ROPE, SCH], BF16)
    agp_in = nc.dram_tensor("agp_in", [1, 16], BF16)
    agp_out = nc.dram_tensor("agp_out", [4, 16], BF16)
    wq = nc.dram_tensor("wq", [E, 6 * 128], DT_PROJ, kind="ExternalInput")
    wkva = nc.dram_tensor("wkva", [E, LORA + ROPE], DT_PROJ, kind="ExternalInput")
    wkvbk = nc.dram_tensor("wkvbk", [LORA, HPC * NOPE], DT_KVB, kind="ExternalInput")
    wkvbv = nc.dram_tensor("wkvbv", [LORA, HPC * VD], DT_KVB, kind="ExternalInput")
    wo = nc.dram_tensor("wo", [HPC * VD, E], DT_WO, kind="ExternalInput")
    kvsc = nc.dram_tensor("kvsc", [128, NC_], F32, kind="ExternalInput")
    cos4 = nc.dram_tensor("cos4", [128, S], BF16, kind="ExternalInput")
    sin4 = nc.dram_tensor("sin4", [128, S], BF16, kind="ExternalInput")
    trilm = nc.dram_tensor("trilm", [128, 128], BF16, kind="ExternalInput")
    out = nc.dram_tensor("out", [E, S], BF16, kind="ExternalOutput")

    with tile.TileContext(nc) as tc:
        with (
            tc.tile_pool(name="persist", bufs=1) as pp,
            tc.tile_pool(name="tables", bufs=1) as tbl,
        ):
            tril_t = tbl.tile([128, 128], BF16)
            kvsc_t = tbl.tile([128, NC_], F32)
            ones_t = tbl.tile([128, 1], BF16)
            onesr_t = tbl.tile([1, 128], F32)
            onesrb_t = tbl.tile([1, 128], BF16)
            eps_t = tbl.tile([1, 1], F32)
            nc.sync.dma_start(tril_t[:], trilm[:])
            nc.sync.dma_start(kvsc_t[:], kvsc[:])
            nc.vector.memset(ones_t[:], 1.0)
            nc.vector.memset(onesr_t[:], 1.0)
            nc.vector.memset(onesrb_t[:], 1.0)
            nc.vector.memset(eps_t[:], EPS)

            # persistent per-chunk activations (precise deps -> overlap)
            q_nope = [[pp.tile([128, SCH], DT_SC, name=f"q_nope{h}_{c}")
                       for c in range(NSC)] for h in range(HPC)]
            # q rope packed 2 heads per tile: rows 0:64 head 2p, 64:128
            # head 2p+1 (each head: lo 32 | hi 32)
            q_ropep = [[pp.tile([128, SCH], DT_SC, name=f"q_ropep{p}_{c}")
                        for c in range(NSC)] for p in range(2)]
            # k rope duplicated rows 0:64 == 64:128 for row-tiled rope MMs
            krd = [pp.tile([128, SCH], DT_SC, name=f"krd{c}")
                   for c in range(NSC)]
            ckv_n = [[pp.tile([128, SCH], DT_KVB, name=f"ckv_n{c}_{s}")
                      for s in range(NSC)] for c in range(NC_)]
            k_nope = [[pp.tile([128, SCH], DT_SC, name=f"k_nope{h}_{c}")
                       for c in range(NSC)] for h in range(HPC)]
            v_t = [[pp.tile([128, 4, VD], DT_V, name=f"v{h}_{c}")
                    for c in range(NSC)] for h in range(HPC)]
            wbk_t = pp.tile([128, NC_, HPC * NOPE], DT_KVB)
            wbv_t = pp.tile([128, NC_, HPC * VD], DT_KVB)

            # ---------- Phase 1: q + kva projections ----------
            with (
                tc.tile_pool(name="p1w", bufs=1) as p1w,
                tc.tile_pool(name="p1rope", bufs=2) as p1r,
                tc.tile_pool(name="p1x", bufs=26) as p1x,
                tc.tile_pool(name="p1tmp", bufs=2) as p1tmp,
                tc.tile_pool(name="ps1", bufs=8, space="PSUM") as psp,
            ):
                wq_t = p1w.tile([128, NE, 6 * 128], DT_PROJ)
                wkva_t = p1w.tile([128, NE, LORA + ROPE], DT_PROJ)
                cos_t = p1w.tile([128, S], BF16)
                sin_t = p1w.tile([128, S], BF16)
                # kva quarter inputs land first so the sharded kva pass can
                # start immediately and trigger the AllGather early
                xqs = [p1x.tile([128, SCH], DT_PROJ, name=f"xqs_{e}",
                                tag="xts") for e in range(NE)]
                for e in range(NE):
                    nc.sync.dma_start(xqs[e][:], xq[e * 128:(e + 1) * 128, :])
                    nc.sync.dma_start(wkva_t[:, e, :],
                                      wkva[e * 128:(e + 1) * 128, :])
                cosq_t = p1w.tile([128, SCH], BF16)
                sinq_t = p1w.tile([128, SCH], BF16)
                nc.sync.dma_start(cosq_t[:], cosq[:])
                nc.sync.dma_start(sinq_t[:], sinq[:])

                # q-projection inputs (first chunk + weights)
                xts0 = [p1x.tile([128, SCH], DT_PROJ, name=f"xts0_{e}",
                                 tag="xts") for e in range(NE)]
                for e in range(NE):
                    nc.sync.dma_start(xts0[e][:], xt[e * 128:(e + 1) * 128,
                                                     0:SCH])
                    nc.sync.dma_start(wq_t[:, e, :],
                                      wq[e * 128:(e + 1) * 128, :])
                nc.sync.dma_start(cos_t[:], cos4[:])
                nc.sync.dma_start(sin_t[:], sin4[:])

                # phase-2 weight preloads (bandwidth is free here)
                nc.sync.dma_start(
                    wbk_t[:], wkvbk.rearrange("(nc p) d -> p nc d", p=128))
                nc.sync.dma_start(
                    wbv_t[:], wkvbv.rearrange("(nc p) d -> p nc d", p=128))

                # ---- sharded kva: this core's s-quarter only ----
                psB = [psp.tile([128, SCH], F32, tag="ps",
                                name=f"psBq_{c}") for c in range(NC_)]
                psK = psp.tile([64, SCH], F32, tag="ps", name="psKq")
                for e in range(NE):
                    for c in range(NC_):
                        nc.tensor.matmul(
                            psB[c][:], wkva_t[:, e, c * 128:(c + 1) * 128],
                            xqs[e][:], start=(e == 0), stop=(e == NE - 1))
                    nc.tensor.matmul(psK[:], wkva_t[:, e, LORA:LORA + ROPE],
                                     xqs[e][:], start=(e == 0),
                                     stop=(e == NE - 1))
                ckv_q = [p1r.tile([128, SCH], BF16, name=f"ckv_q{c}",
                                  tag=f"ckvq{c}", bufs=1) for c in range(NC_)]
                for c in range(NC_):
                    nc.scalar.copy(ckv_q[c][:], psB[c][:])
                kpe_q = p1r.tile([64, SCH], BF16, tag="kpeq", bufs=1)
                nc.scalar.copy(kpe_q[:], psK[:])
                # RMSNorm on the quarter
                sqq = [p1tmp.tile([128, SCH], BF16, name=f"sqq{c}",
                                  tag=f"sqq{c}", bufs=1) for c in range(NC_)]
                for c in range(NC_):
                    nc.vector.tensor_tensor(sqq[c][:], ckv_q[c][:], ckv_q[c][:],
                                            mybir.AluOpType.mult)
                # first q-projection e-slice keeps the PE busy while the
                # RMSNorm square chain (ACT+DVE) produces sqq
                psA0 = [psp.tile([128, SCH], F32, tag="ps",
                                 name=f"psA0_{d}") for d in range(6)]
                for d in range(6):
                    nc.tensor.matmul(
                        psA0[d][:], wq_t[:, 0, d * 128:(d + 1) * 128],
                        xts0[0][:], start=True, stop=False)
                pss = psp.tile([1, SCH], F32, tag="ps", name="ssqq")
                for c in range(NC_):
                    nc.tensor.matmul(pss[:], ones_t[:], sqq[c][:],
                                     start=(c == 0), stop=(c == NC_ - 1))
                s_rowq = p1tmp.tile([1, SCH], F32, tag="srq", bufs=1)
                r_rowq = p1tmp.tile([1, SCH], F32, tag="rrq", bufs=1)
                nc.scalar.activation(s_rowq[:], pss[:],
                                     mybir.ActivationFunctionType.Ln,
                                     bias=eps_t[:], scale=1.0 / LORA)
                nc.scalar.activation(r_rowq[:], s_rowq[:],
                                     mybir.ActivationFunctionType.Exp,
                                     scale=-0.5)
                r_bcq = psp.tile([128, SCH], F32, tag="ps", name="rbcq")
                nc.tensor.matmul(r_bcq[:], onesr_t[0:1, :], r_rowq[:],
                                 start=True, stop=True)
                ckv_nq = [p1r.tile([128, SCH], BF16, name=f"ckv_nq{c}",
                                   tag=f"ckvnq{c}", bufs=1)
                          for c in range(NC_)]
                for c in range(NC_):
                    nc.vector.scalar_tensor_tensor(
                        ckv_nq[c][:], ckv_q[c][:], kvsc_t[:, c:c + 1],
                        r_bcq[:], op0=mybir.AluOpType.mult,
                        op1=mybir.AluOpType.mult)
                    nc.sync.dma_start(ccin[c * 128:(c + 1) * 128, :],
                                      ckv_nq[c][:])
                # k_pe rope on the quarter
                kloq = p1r.tile([32, SCH], BF16, tag="kloq", bufs=1)
                khiq = p1r.tile([32, SCH], BF16, tag="khiq", bufs=1)
                nc.sync.dma_start(kloq[:], kpe_q[0:32, :])
                nc.sync.dma_start(khiq[:], kpe_q[32:64, :])
                kt1q = p1tmp.tile([32, SCH], BF16, tag="kt1", bufs=1)
                kt2q = p1tmp.tile([32, SCH], BF16, tag="kt2", bufs=1)
                klorq = p1r.tile([32, SCH], BF16, tag="klorq", bufs=1)
                khirq = p1r.tile([32, SCH], BF16, tag="khirq", bufs=1)
                nc.vector.tensor_tensor(kt1q[:], kloq[:], cosq_t[0:32, :],
                                        mybir.AluOpType.mult)
                nc.vector.tensor_tensor(kt2q[:], khiq[:], sinq_t[0:32, :],
                                        mybir.AluOpType.mult)
                nc.vector.tensor_tensor(klorq[:], kt1q[:], kt2q[:],
                                        mybir.AluOpType.subtract)
                nc.vector.tensor_tensor(kt1q[:], khiq[:], cosq_t[0:32, :],
                                        mybir.AluOpType.mult)
                nc.vector.tensor_tensor(kt2q[:], kloq[:], sinq_t[0:32, :],
                                        mybir.AluOpType.mult)
                nc.vector.tensor_tensor(khirq[:], kt1q[:], kt2q[:],
                                        mybir.AluOpType.add)
                nc.sync.dma_start(ccin[LORA:LORA + 32, :], klorq[:])
                nc.sync.dma_start(ccin[LORA + 32:LORA + ROPE, :], khirq[:])
                nc.gpsimd.collective_compute(
                    "AllGather", mybir.AluOpType.bypass,
                    replica_groups=[[0, 1, 2, 3], [4, 5, 6, 7]],
                    ins=[ccin[:]], outs=[ccgat[:]])

                for sc in range(NSC):
                    ssl = bass.ts(sc, SCH)
                    if sc == 0:
                        xts = xts0
                        psA = psA0
                        e0 = 1  # e == 0 already issued above
                    else:
                        xts = [p1x.tile([128, SCH], DT_PROJ,
                                        name=f"xts{sc}_{e}", tag="xts")
                               for e in range(NE)]
                        for e in range(NE):
                            nc.sync.dma_start(
                                xts[e][:], xt[e * 128:(e + 1) * 128, ssl])
                        psA = [psp.tile([128, SCH], F32, tag="ps",
                                        name=f"psA{sc}_{d}") for d in range(6)]
                        e0 = 0
                    # pass A: 6 q-projection groups, e-outer
                    for e in range(e0, NE):
                        for d in range(6):
                            nc.tensor.matmul(
                                psA[d][:], wq_t[:, e, d * 128:(d + 1) * 128],
                                xts[e][:], start=(e == 0), stop=(e == NE - 1))
                    for h in range(HPC):
                        nc.scalar.copy(q_nope[h][sc][:], psA[h][:])
                    # drain rope psum banks to SBUF first (ACT) so the next
                    # chunk's psA accumulation can reuse them promptly
                    lo_sb = p1tmp.tile([128, SCH], BF16, tag="losb",
                                       name=f"losb_{sc}")
                    hi_sb = p1tmp.tile([128, SCH], BF16, tag="hisb",
                                       name=f"hisb_{sc}")
                    nc.scalar.copy(lo_sb[:], psA[4][:])
                    nc.scalar.copy(hi_sb[:], psA[5][:])
                    # q rope: lo' = lo*cos - hi*sin ; hi' = hi*cos + lo*sin
                    t1 = p1tmp.tile([128, SCH], BF16, tag="t1")
                    t2 = p1tmp.tile([128, SCH], BF16, tag="t2")
                    t3 = p1tmp.tile([128, SCH], BF16, tag="t1",
                                    name=f"t3_{sc}")
                    t4 = p1tmp.tile([128, SCH], BF16, tag="t2",
                                    name=f"t4_{sc}")
                    qlo4 = p1r.tile([128, SCH], BF16, tag="qlo4",
                                    name=f"qlo4_{sc}")
                    qhi4 = p1r.tile([128, SCH], BF16, tag="qhi4",
                                    name=f"qhi4_{sc}")
                    nc.vector.tensor_tensor(t1[:], lo_sb[:], cos_t[:, ssl],
                                            mybir.AluOpType.mult)
                    nc.vector.tensor_tensor(t2[:], hi_sb[:], sin_t[:, ssl],
                                            mybir.AluOpType.mult)
                    nc.vector.tensor_tensor(qlo4[:], t1[:], t2[:],
                                            mybir.AluOpType.subtract)
                    nc.vector.tensor_tensor(t3[:], hi_sb[:], cos_t[:, ssl],
                                            mybir.AluOpType.mult)
                    nc.vector.tensor_tensor(t4[:], lo_sb[:], sin_t[:, ssl],
                                            mybir.AluOpType.mult)
                    nc.vector.tensor_tensor(qhi4[:], t3[:], t4[:],
                                            mybir.AluOpType.add)
                    for p in range(2):
                        for j in range(2):
                            h = 2 * p + j
                            hs = bass.ts(h, 32)
                            nc.sync.dma_start(
                                q_ropep[p][sc][64 * j:64 * j + 32, :],
                                qlo4[hs, :])
                            nc.sync.dma_start(
                                q_ropep[p][sc][64 * j + 32:64 * j + 64, :],
                                qhi4[hs, :])

                # pull the gathered, normalized ckv + roped k_pe into SBUF
                # (persistent tiles: transfers overlap the q projection)
                for sc in range(NSC):
                    for c in range(NC_):
                        nc.sync.dma_start(ckv_n[c][sc][:],
                                          ccgat[sc, c * 128:(c + 1) * 128, :])
                    nc.sync.dma_start(krd[sc][0:64, :],
                                      ccgat[sc, LORA:LORA + ROPE, :])
                    nc.sync.dma_start(krd[sc][64:128, :],
                                      ccgat[sc, LORA:LORA + ROPE, :])

            # ---------- fused per-chunk pipeline: kvb / attn / wo ----------
            with (
                tc.tile_pool(name="p2w", bufs=1) as p2w,
                tc.tile_pool(name="probs", bufs=8) as prp,
                tc.tile_pool(name="acc", bufs=4) as accp,
                tc.tile_pool(name="att2", bufs=8) as attp,
                tc.tile_pool(name="a2tmp", bufs=4) as a2t,
                tc.tile_pool(name="aout", bufs=4) as aout,
                tc.tile_pool(name="psO", bufs=4, space="PSUM") as psO,
                tc.tile_pool(name="psS", bufs=4, space="PSUM") as psS,
            ):
                wo_t = [p2w.tile([128, E], DT_WO, name=f"wo_t{h}")
                        for h in range(HPC)]
                for h in range(HPC):
                    nc.sync.dma_start(wo_t[h][:], wo[h * 128:(h + 1) * 128, :])
                def emit_kvb(qc):
                    # kvb(qc): k_nope + V (drains on DVE, keeping ACT free
                    # for the endgame's copies at the chunk boundary)
                    for h in range(HPC):
                        ps2 = psS.tile([128, SCH], F32, tag="sc",
                                       name=f"kvbk{qc}_{h}")
                        for c in range(NC_):
                            nc.tensor.matmul(
                                ps2[:], wbk_t[:, c, h * 128:(h + 1) * 128],
                                ckv_n[c][qc][:], start=(c == 0),
                                stop=(c == NC_ - 1))
                        nc.vector.tensor_copy(k_nope[h][qc][:], ps2[:])
                    for sbl in range(4):
                        psv = psS.tile([128, HPC * VD], F32, tag="sc",
                                       name=f"v{qc}_{sbl}")
                        for c in range(NC_):
                            nc.tensor.matmul(
                                psv[:], ckv_n[c][qc][:, bass.ts(sbl, 128)],
                                wbv_t[:, c, :], start=(c == 0),
                                stop=(c == NC_ - 1))
                        for h in range(HPC):
                            nc.vector.tensor_copy(v_t[h][qc][:, sbl, :],
                                                  psv[:, h * VD:(h + 1) * VD])

                emit_kvb(0)
                for qc in range(NSC):
                    # ---- attention(qc), two heads per pass ----
                    # AV matmuls run one ki behind the score matmuls so the
                    # PE never waits on the exp; denominators accumulate on
                    # DVE (heads 0,2) / GpSimd (heads 1,3); all four heads'
                    # normalization chains are batched after both passes.
                    nki = 4 * qc + 4
                    att2 = [None] * HPC
                    out2h = [None] * HPC
                    acch = [None] * HPC
                    for p in range(2):
                        hA, hB = 2 * p, 2 * p + 1
                        out2 = [psO.tile([128, SCH], F32, tag="o2",
                                         name=f"o2_{qc}_{p}_{j}")
                                for j in range(2)]
                        acc = [accp.tile([128, SCH], F32, tag="acc",
                                         name=f"acc{qc}_{p}_{j}")
                               for j in range(2)]
                        out2h[hA], out2h[hB] = out2[0], out2[1]
                        acch[hA], acch[hB] = acc[0], acc[1]

                        def emit_av(ent):
                            pki, pvl, pprs = ent
                            pkc, pko = pki // 4, pki % 4
                            for j, hh in enumerate((hA, hB)):
                                nc.tensor.matmul(
                                    out2[j][:, pvl], v_t[hh][pkc][:, pko, :],
                                    pprs[j][:, pvl], start=(pki == 0),
                                    stop=(pki == nki - 1))

                        # AV runs two ki behind scores so the PE never waits
                        # on the exp (ACT runs ~1 ki behind with slack)
                        pend = []
                        for ki in range(nki):
                            kc, ko = ki // 4, ki % 4
                            js = max(0, ki - 4 * qc)
                            w = SCH - js * 128
                            vl = bass.ds(js * 128, w)
                            kb = bass.ts(ko, 128)
                            ps2h = [psS.tile([128, SCH], F32, tag="sc",
                                             name=f"sc{qc}_{p}_{ki}_{j}")
                                    for j in range(2)]
                            for j, hh in enumerate((hA, hB)):
                                nc.tensor.matmul(
                                    ps2h[j][:, vl], k_nope[hh][kc][:, kb],
                                    q_nope[hh][qc][:, vl],
                                    start=True, stop=False)
                            # rope: both heads concurrently via row tiling
                            nc.tensor.matmul(
                                ps2h[0][:, vl], krd[kc][0:64, kb],
                                q_ropep[p][qc][0:64, vl],
                                start=False, stop=True)
                            nc.tensor.matmul(
                                ps2h[1][:, vl], krd[kc][64:128, kb],
                                q_ropep[p][qc][64:128, vl],
                                start=False, stop=True)
                            if len(pend) == 2:
                                emit_av(pend.pop(0))
                            prs = []
                            for j, hh in enumerate((hA, hB)):
                                pr = prp.tile([128, SCH], DT_V, tag="pr",
                                              name=f"pr{qc}_{p}_{ki}_{j}")
                                nc.scalar.activation(
                                    pr[:, vl], ps2h[j][:, vl],
                                    mybir.ActivationFunctionType.Exp,
                                    scale=SM_SCALE)
                                if ki >= 4 * qc:
                                    jb = bass.ts(ki - 4 * qc, 128)
                                    nc.vector.tensor_tensor(
                                        pr[:, jb], pr[:, jb], tril_t[:],
                                        mybir.AluOpType.mult)
                                # denominator running sum (f32, DVE)
                                if ki == 0:
                                    nc.vector.tensor_copy(acc[j][:], pr[:])
                                else:
                                    nc.vector.tensor_tensor(
                                        acc[j][:, vl], acc[j][:, vl],
                                        pr[:, vl], mybir.AluOpType.add)
                                prs.append(pr)
                            pend.append((ki, vl, prs))
                        for ent in pend:
                            emit_av(ent)
                        # bf16 denominator rows for this pair (DVE-only, no
                        # PE involvement, lands early in the DVE queue)
                        for j, hh in enumerate((hA, hB)):
                            accb = a2t.tile([128, SCH], BF16, tag="accb",
                                            name=f"accb{qc}_{hh}")
                            nc.vector.tensor_copy(accb[:], acc[j][:])
                            acch[hh] = accb
                    accbs = acch
                    # next chunk's kvb: PE filler while the denominator
                    # chains drain on DVE
                    if qc + 1 < NSC:
                        emit_kvb(qc + 1)
                    # normalize: att2 = out2 / colsum (all heads batched)
                    recs, psds, psbcs, bcss = [], [], [], []
                    for hh in range(HPC):
                        psd = psS.tile([1, SCH], F32, tag="sc",
                                       padded_shape=[128, SCH],
                                       name=f"psd{qc}_{hh}")
                        nc.tensor.matmul(psd[:], ones_t[:], accbs[hh][:],
                                         start=True, stop=True)
                        psds.append(psd)
                    recfs = []
                    for hh in range(HPC):
                        recf = a2t.tile([1, SCH], F32, tag="recf",
                                        name=f"recf{qc}_{hh}")
                        nc.vector.reciprocal_approx_fast(recf[:],
                                                         psds[hh][:])
                        recfs.append(recf)
                    for hh in range(HPC):
                        rec = a2t.tile([1, SCH], BF16, tag="rec",
                                       name=f"rec{qc}_{hh}")
                        nc.scalar.copy(rec[:], recfs[hh][:])
                        recs.append(rec)
                    for hh in range(HPC):
                        psbc = psS.tile([128, SCH], F32, tag="sc",
                                        name=f"psbc{qc}_{hh}")
                        nc.tensor.matmul(psbc[:], onesrb_t[0:1, :],
                                         recs[hh][:], start=True, stop=True)
                        psbcs.append(psbc)
                    for hh in range(HPC):
                        bcs = a2t.tile([128, SCH], BF16, tag="bcs",
                                       name=f"bcs{qc}_{hh}")
                        nc.scalar.copy(bcs[:], psbcs[hh][:])
                        bcss.append(bcs)
                    for hh in range(HPC):
                        at2 = attp.tile([128, SCH], DT_WO, tag="att2",
                                        name=f"att2_{qc}_{hh}")
                        nc.vector.tensor_tensor(at2[:], out2h[hh][:],
                                                bcss[hh][:],
                                                mybir.AluOpType.mult)
                        att2[hh] = at2

                    # ---- wo(qc): outT[e, q] += wo^T @ att2 ----
                    for ecg in range(4):
                        psos = [psS.tile([128, SCH], F32, tag="sc",
                                         name=f"po{qc}_{ecg}_{j}")
                                for j in range(4)]
                        for h in range(HPC):
                            for j in range(4):
                                ec = 4 * ecg + j
                                nc.tensor.matmul(
                                    psos[j][:], wo_t[h][:, bass.ts(ec, 128)],
                                    att2[h][:], start=(h == 0),
                                    stop=(h == HPC - 1))
                        for j in range(4):
                            ec = 4 * ecg + j
                            ot = aout.tile([128, SCH], BF16, tag="ot",
                                           name=f"ot{qc}_{ec}")
                            if j % 2 == 0:
                                nc.scalar.copy(ot[:], psos[j][:])
                            else:
                                nc.vector.tensor_copy(ot[:], psos[j][:])
                            nc.sync.dma_start(
                                out[bass.ts(ec, 128), bass.ts(qc, SCH)],
                                ot[:])

    nc.finalize()
    return nc


def _prep_inputs(x, wq, wkv_a, wkv_b, wo, kv_norm_scale):
    """Build the 8 per-core input dicts (numpy, host-side sharding)."""
    x = np.asarray(x, np.float32)
    wq = np.asarray(wq, np.float32)
    wkv_a = np.asarray(wkv_a, np.float32)
    wkv_b = np.asarray(wkv_b, np.float32)
    wo = np.asarray(wo, np.float32)
    kv_norm_scale = np.asarray(kv_norm_scale, np.float32)

    bf = ml_dtypes.bfloat16
    pos = np.arange(S, dtype=np.float32)
    inv = 1.0 / (ROPE_THETA ** (np.arange(0, ROPE, 2, dtype=np.float32) / ROPE))
    ang = pos[:, None] * inv  # [S, 32]
    cosT = np.cos(ang).T  # [32, S]
    sinT = np.sin(ang).T
    cos4 = np.tile(cosT, (4, 1)).astype(bf)
    sin4 = np.tile(sinT, (4, 1)).astype(bf)
    tril = (np.arange(128)[None, :] >= np.arange(128)[:, None]).astype(bf)
    kvsc = kv_norm_scale.reshape(NC_, 128).T.copy()  # [128, NC_]

    wq_r = wq.reshape(E, H, QH)
    wkv_b_r = wkv_b.reshape(LORA, H, NOPE + VD)
    wo_r = wo.reshape(H, VD, E)

    in_maps = []
    for c in range(N_CORES):
        b, hg = c // 4, c % 4
        hs = [4 * hg + j for j in range(HPC)]
        xt = np.ascontiguousarray(x[b].T).astype(NP_OF[DT_PROJ])
        r = hg  # this core's s-quarter for the sharded kva projection
        xq_loc = np.ascontiguousarray(xt[:, r * SCH:(r + 1) * SCH])
        cosq_loc = np.ascontiguousarray(cos4[:, r * SCH:(r + 1) * SCH])
        sinq_loc = np.ascontiguousarray(sin4[:, r * SCH:(r + 1) * SCH])
        # wq cols: nope h0..h3 | lo4 | hi4
        wq_loc = np.concatenate(
            [wq_r[:, h, 0:NOPE] for h in hs]
            + [np.concatenate([wq_r[:, h, NOPE:NOPE + 32] for h in hs], axis=1)]
            + [np.concatenate([wq_r[:, h, NOPE + 32:QH] for h in hs], axis=1)],
            axis=1).astype(NP_OF[DT_PROJ])
        wkvbk = np.concatenate([wkv_b_r[:, h, 0:NOPE] for h in hs],
                               axis=1).astype(NP_OF[DT_KVB])
        wkvbv = np.concatenate([wkv_b_r[:, h, NOPE:] for h in hs],
                               axis=1).astype(NP_OF[DT_KVB])
        wo_loc = np.concatenate([wo_r[h] for h in hs],
                                axis=0).astype(NP_OF[DT_WO])
        in_maps.append({
            "xt": xt,
            "xq": xq_loc,
            "cosq": cosq_loc,
            "sinq": sinq_loc,
            "wq": wq_loc,
            "wkva": wkv_a.astype(NP_OF[DT_PROJ]),
            "wkvbk": wkvbk,
            "wkvbv": wkvbv,
            "wo": wo_loc,
            "kvsc": kvsc,
            "cos4": cos4,
            "sin4": sin4,
            "trilm": tril,
        })
    return in_maps


_LAST_EXEC_NS = None


def kernel(x, wq, wkv_a, wkv_b, wo, kv_norm_scale, _trace=False):
    global _LAST_EXEC_NS
    nc = build_nc()
    in_maps = _prep_inputs(x, wq, wkv_a, wkv_b, wo, kv_norm_scale)
    res = run_bass_kernel_spmd(nc, in_maps, list(range(N_CORES)), trace=_trace)
    _LAST_EXEC_NS = res.exec_time_ns
    out = np.zeros((B, S, E), np.float32)
    for c in range(N_CORES):
        out[c // 4] += np.asarray(res.results[c]["out"],
                                  dtype=np.float32).T
    return out
